# revision 18
# baseline (speedup 1.0000x reference)
"""TRN2 Bass kernel: relu + per-row top-32 masking for x [4096, 32768] f32.

kernel(x) -> (relu(x), topk_masked) matching:
    y = relu(x); vals, idx = top_k(y, 32); xz = zeros.at[rows, idx].set(vals)

Sharding: pure data parallel over rows, 8 NeuronCores x [512, 32768].

Per-core algorithm (exact for any input with >=32 positive entries per row):
  stream x in column sub-tiles: relu on ScalarE -> write y; chunk maxes
  (256 chunks of 128) on VectorE. Top-32 chunks via 4 rounds of DVE
  max8 + match_replace on a copy (selection mask = work != orig, which
  reproduces stable lower-index tie-breaking exactly); compact selected
  chunk ids by max8-extracting (256-c)*sel; indirect-DMA gather those 32
  chunks/row from DRAM; same max8 trick picks the top-32 elements; the
  masked chunks (value * keep) are indirect-DMA scattered back into the
  pre-zeroed xz output, so only ~1/8 of xz is ever written.
"""

import os
import sys

if "/opt/trn_rl_repo" not in sys.path:
    sys.path.insert(0, "/opt/trn_rl_repo")

import numpy as np

import concourse.bass as bass
import concourse.mybir as mybir
from concourse import bacc
from concourse.bass_utils import run_bass_kernel_spmd
from concourse.tile import TileContext

F32 = mybir.dt.float32
I32 = mybir.dt.int32
I16 = mybir.dt.int16

N_ROWS = 4096
N_COLS = 32768
N_CORES = 8
L = 128          # chunk length
K = 32           # top-k
P = 128          # rows per block (partitions)

LAST_EXEC_TIME_NS = None
LAST_TRACE_DIR = None
_CACHED_NC = None


def _build(R: int, D: int, sub: int = 4096, g_bufs: int = 2, x_bufs: int = 4,
           m_bufs: int = 3, s_bufs: int = 4):
    C = D // L
    n_blocks = R // P
    n_sub = D // sub
    sub_chunks = sub // L

    nc = bacc.Bacc("TRN2", target_bir_lowering=False, debug=False)
    x = nc.declare_dram_parameter("x", [R, D], F32, isOutput=False)
    y = nc.declare_dram_parameter("y", [R, D], F32, isOutput=True)
    xz = nc.declare_dram_parameter("xz", [R, D], F32, isOutput=True)

    x_chunks = x[:].rearrange("r (c l) -> (r c) l", l=L)
    xz_chunks = xz[:].rearrange("r (c l) -> (r c) l", l=L)

    with TileContext(nc) as tc:
        with (
            tc.tile_pool(name="consts", bufs=1) as const_pool,
            tc.tile_pool(name="xstream", bufs=x_bufs) as x_pool,
            tc.tile_pool(name="mstage", bufs=m_bufs) as m_pool,
            tc.tile_pool(name="gstage", bufs=g_bufs) as g_pool,
            tc.tile_pool(name="small", bufs=s_bufs) as s_pool,
        ):
            # ids_iota[p, c] = C - c  (max8 extraction then yields ascending chunk id)
            ids_iota_i = const_pool.tile([P, C], I32, tag="ids_iota_i")
            nc.gpsimd.iota(ids_iota_i[:], pattern=[[-1, C]], base=C, channel_multiplier=0)
            ids_iota = const_pool.tile([P, C], F32, tag="ids_iota")
            nc.vector.tensor_copy(ids_iota[:], ids_iota_i[:])
            # rowbase[p, 0] = p*C + C  (global chunk = rowbase - e + block_base)
            rowbase_i = const_pool.tile([P, 1], I32, tag="rowbase_i")
            nc.gpsimd.iota(rowbase_i[:], pattern=[[0, 1]], base=C, channel_multiplier=C)
            rowbase = const_pool.tile([P, 1], F32, tag="rowbase")
            nc.vector.tensor_copy(rowbase[:], rowbase_i[:])

            # deferred scatter state: emit scatters of block b-1 after the
            # gathers of block b so the Pool engine's in-order instruction
            # stream never stalls on block b-1's DVE G-stage.
            pending_scatter = None

            def emit_scatter(state):
                sb, s_offs, s_Gw3 = state
                for k in range(K):
                    nc.gpsimd.indirect_dma_start(
                        out=xz_chunks,
                        out_offset=bass.IndirectOffsetOnAxis(ap=s_offs[:, k:k + 1], axis=0),
                        in_=s_Gw3[:, k, :],
                        in_offset=None,
                    )

            for b in range(n_blocks):
                r0 = b * P
                M = m_pool.tile([P, C], F32, tag="M")
                for s in range(n_sub):
                    c0 = s * sub
                    xt = x_pool.tile([P, sub], F32, tag="xt")
                    nc.sync.dma_start(out=xt[:], in_=x[r0:r0 + P, c0:c0 + sub])
                    nc.scalar.activation(xt[:], xt[:], mybir.ActivationFunctionType.Relu)
                    nc.sync.dma_start(out=y[r0:r0 + P, c0:c0 + sub], in_=xt[:])
                    nc.vector.tensor_reduce(
                        out=M[:, s * sub_chunks:(s + 1) * sub_chunks],
                        in_=xt[:].rearrange("p (c l) -> p c l", l=L),
                        axis=mybir.AxisListType.X,
                        op=mybir.AluOpType.max,
                    )

                # top-32 chunks (first round reads M directly, rest in-place on Mw)
                Mw = m_pool.tile([P, C], F32, tag="Mw")
                mx8 = s_pool.tile([P, 8], F32, tag="mx8")
                src = M
                for _ in range(K // 8):
                    nc.vector.max(mx8[:], src[:])
                    nc.vector.match_replace(out=Mw[:], in_to_replace=mx8[:],
                                            in_values=src[:], imm_value=-1.0)
                    src = Mw
                selM = m_pool.tile([P, C], F32, tag="selM")
                nc.vector.tensor_tensor(out=selM[:], in0=Mw[:], in1=M[:],
                                        op=mybir.AluOpType.not_equal)
                ids = m_pool.tile([P, C], F32, tag="ids")
                nc.vector.tensor_tensor(out=ids[:], in0=selM[:], in1=ids_iota[:],
                                        op=mybir.AluOpType.mult)
                idsel = s_pool.tile([P, K], F32, tag="idsel")
                for r in range(K // 8):
                    nc.vector.max(idsel[:, r * 8:(r + 1) * 8], ids[:])
                    nc.vector.match_replace(out=ids[:], in_to_replace=idsel[:, r * 8:(r + 1) * 8],
                                            in_values=ids[:], imm_value=0.0)

                # offsets: global chunk index = rowbase - e + b*P*C (exact in f32)
                offs_f = s_pool.tile([P, K], F32, tag="offs_f")
                nc.vector.tensor_scalar(offs_f[:], idsel[:], -1.0, None,
                                        op0=mybir.AluOpType.mult)
                nc.vector.tensor_scalar(offs_f[:], offs_f[:], rowbase[:, :1],
                                        float(b * P * C),
                                        op0=mybir.AluOpType.add, op1=mybir.AluOpType.add)
                offs = s_pool.tile([P, K], I32, tag="offs")
                nc.vector.tensor_copy(offs[:], offs_f[:])

                # gather 32 chunks/row from x. The walrus indirect-DMA lowering
                # supports one dynamic offset per partition and a 2D SBUF side,
                # so issue K gathers of [P, L] each.
                G = g_pool.tile([P, K, L], F32, tag="G")
                for k in range(K):
                    nc.gpsimd.indirect_dma_start(
                        out=G[:, k, :], out_offset=None,
                        in_=x_chunks,
                        in_offset=bass.IndirectOffsetOnAxis(ap=offs[:, k:k + 1], axis=0),
                    )
                if pending_scatter is not None:
                    emit_scatter(pending_scatter)
                    pending_scatter = None
                Gf = G[:].rearrange("p k l -> p (k l)")

                # top-32 elements of G: zap to 0, then masked = Gf - Gw
                # (kept values are > 0 whenever every row has >= 32 positives,
                #  so zapped zeros are never re-extracted)
                Gw = g_pool.tile([P, K * L], F32, tag="Gw")
                gx8 = s_pool.tile([P, 8], F32, tag="gx8")
                gsrc = Gf
                for _ in range(K // 8):
                    nc.vector.max(gx8[:], gsrc)
                    nc.vector.match_replace(out=Gw[:], in_to_replace=gx8[:],
                                            in_values=gsrc, imm_value=0.0)
                    gsrc = Gw[:]
                nc.vector.tensor_tensor(out=Gw[:], in0=Gf, in1=Gw[:],
                                        op=mybir.AluOpType.subtract)

                # scatter masked chunks into pre-zeroed xz, K scatters of [P, L];
                # deferred to after the next block's gathers (see above).
                pending_scatter = (b, offs, Gw[:].rearrange("p (k l) -> p k l", l=L))
            if pending_scatter is not None:
                emit_scatter(pending_scatter)
    nc.finalize()
    return nc


def kernel(x: np.ndarray):
    global LAST_EXEC_TIME_NS, LAST_TRACE_DIR, _CACHED_NC
    x = np.ascontiguousarray(np.asarray(x, dtype=np.float32))
    assert x.shape == (N_ROWS, N_COLS), x.shape
    Rs = N_ROWS // N_CORES

    if _CACHED_NC is None:
        _CACHED_NC = _build(Rs, N_COLS)
    nc = _CACHED_NC

    in_maps = [{"x": x[i * Rs:(i + 1) * Rs]} for i in range(N_CORES)]
    tmpdir = None
    if os.environ.get("BASS_TRACE"):
        import tempfile
        tmpdir = tempfile.mkdtemp(prefix="topk_trace_")
        LAST_TRACE_DIR = tmpdir
    res = run_bass_kernel_spmd(nc, in_maps, core_ids=list(range(N_CORES)),
                               tmpdir=tmpdir)
    LAST_EXEC_TIME_NS = res.exec_time_ns

    y = np.concatenate([np.asarray(res.results[i]["y"]).reshape(Rs, N_COLS)
                        for i in range(N_CORES)], axis=0)
    xz = np.concatenate([np.asarray(res.results[i]["xz"]).reshape(Rs, N_COLS)
                         for i in range(N_CORES)], axis=0)
    return y, xz


# revision 19
# speedup vs baseline: 1.1904x; 1.1904x over previous
"""TRN2 Bass kernel: relu + per-row top-32 masking for x [4096, 32768] f32.

kernel(x) -> (relu(x), topk_masked) matching:
    y = relu(x); vals, idx = top_k(y, 32); xz = zeros.at[rows, idx].set(vals)

Sharding: pure data parallel over rows, 8 NeuronCores x [512, 32768].

Per-core algorithm (exact for any input with >=32 positive entries per row):
  stream x in column sub-tiles: relu on ScalarE -> write y; chunk maxes
  (256 chunks of 128) on VectorE. Top-32 chunks via 4 rounds of DVE
  max8 + match_replace on a copy (selection mask = work != orig, which
  reproduces stable lower-index tie-breaking exactly); compact selected
  chunk ids by max8-extracting (256-c)*sel; indirect-DMA gather those 32
  chunks/row from DRAM; same max8 trick picks the top-32 elements; the
  masked chunks (value * keep) are indirect-DMA scattered back into the
  pre-zeroed xz output, so only ~1/8 of xz is ever written.
"""

import os
import sys

if "/opt/trn_rl_repo" not in sys.path:
    sys.path.insert(0, "/opt/trn_rl_repo")

import numpy as np

import concourse.bass as bass
import concourse.mybir as mybir
from concourse import bacc
from concourse.bass_utils import run_bass_kernel_spmd
from concourse.tile import TileContext

F32 = mybir.dt.float32
I32 = mybir.dt.int32
I16 = mybir.dt.int16

N_ROWS = 4096
N_COLS = 32768
N_CORES = 8
L = 128          # chunk length
K = 32           # top-k
P = 128          # rows per block (partitions)

LAST_EXEC_TIME_NS = None
LAST_TRACE_DIR = None
_CACHED_NC = None


def _build(R: int, D: int, sub: int = 4096, g_bufs: int = 2, x_bufs: int = 4,
           m_bufs: int = 3, s_bufs: int = 4):
    C = D // L
    n_blocks = R // P
    n_sub = D // sub
    sub_chunks = sub // L

    nc = bacc.Bacc("TRN2", target_bir_lowering=False, debug=False)
    x = nc.declare_dram_parameter("x", [R, D], F32, isOutput=False)
    y = nc.declare_dram_parameter("y", [R, D], F32, isOutput=True)
    xz = nc.declare_dram_parameter("xz", [R, D], F32, isOutput=True)

    x_chunks = x[:].rearrange("r (c l) -> (r c) l", l=L)
    xz_chunks = xz[:].rearrange("r (c l) -> (r c) l", l=L)

    with TileContext(nc) as tc:
        with (
            tc.tile_pool(name="consts", bufs=1) as const_pool,
            tc.tile_pool(name="xstream", bufs=x_bufs) as x_pool,
            tc.tile_pool(name="mstage", bufs=m_bufs) as m_pool,
            tc.tile_pool(name="gstage", bufs=g_bufs) as g_pool,
            tc.tile_pool(name="small", bufs=s_bufs) as s_pool,
        ):
            # ids_iota[p, c] = C - c  (max8 extraction then yields ascending chunk id)
            ids_iota_i = const_pool.tile([P, C], I32, tag="ids_iota_i")
            nc.gpsimd.iota(ids_iota_i[:], pattern=[[-1, C]], base=C, channel_multiplier=0)
            ids_iota = const_pool.tile([P, C], F32, tag="ids_iota")
            nc.vector.tensor_copy(ids_iota[:], ids_iota_i[:])
            # rowbase[p, 0] = p*C + C  (global chunk = rowbase - e + block_base)
            rowbase_i = const_pool.tile([P, 1], I32, tag="rowbase_i")
            nc.gpsimd.iota(rowbase_i[:], pattern=[[0, 1]], base=C, channel_multiplier=C)
            rowbase = const_pool.tile([P, 1], F32, tag="rowbase")
            nc.vector.tensor_copy(rowbase[:], rowbase_i[:])

            # deferred scatter state: emit scatters of block b-1 after the
            # gathers of block b so the Pool engine's in-order instruction
            # stream never stalls on block b-1's DVE G-stage.
            pending_scatter = None
            # All scatters write the same full-tensor xz AP (the indirect side
            # must have offset 0), so Tile chains them with WAW completion
            # semaphores. The actual chunk destinations are provably disjoint
            # (distinct chunks per row, distinct rows per block), so strip
            # scatter->scatter deps.
            scatter_names = set()

            def emit_scatter(state):
                sb, s_offs, s_Gw3 = state
                for k in range(K):
                    ins = nc.gpsimd.indirect_dma_start(
                        out=xz_chunks,
                        out_offset=bass.IndirectOffsetOnAxis(ap=s_offs[:, k:k + 1], axis=0),
                        in_=s_Gw3[:, k, :],
                        in_offset=None,
                    )
                    ins = getattr(ins, "ins", ins)
                    for dep in list(ins.sync_dependency_names()):
                        if dep in scatter_names:
                            ins.try_remove_dependency(dep)
                    scatter_names.add(ins.name)

            for b in range(n_blocks):
                r0 = b * P
                M = m_pool.tile([P, C], F32, tag="M")
                for s in range(n_sub):
                    c0 = s * sub
                    xt = x_pool.tile([P, sub], F32, tag="xt")
                    nc.sync.dma_start(out=xt[:], in_=x[r0:r0 + P, c0:c0 + sub])
                    nc.scalar.activation(xt[:], xt[:], mybir.ActivationFunctionType.Relu)
                    nc.sync.dma_start(out=y[r0:r0 + P, c0:c0 + sub], in_=xt[:])
                    nc.vector.tensor_reduce(
                        out=M[:, s * sub_chunks:(s + 1) * sub_chunks],
                        in_=xt[:].rearrange("p (c l) -> p c l", l=L),
                        axis=mybir.AxisListType.X,
                        op=mybir.AluOpType.max,
                    )

                # top-32 chunks (first round reads M directly, rest in-place on Mw)
                Mw = m_pool.tile([P, C], F32, tag="Mw")
                mx8 = s_pool.tile([P, 8], F32, tag="mx8")
                src = M
                for _ in range(K // 8):
                    nc.vector.max(mx8[:], src[:])
                    nc.vector.match_replace(out=Mw[:], in_to_replace=mx8[:],
                                            in_values=src[:], imm_value=-1.0)
                    src = Mw
                selM = m_pool.tile([P, C], F32, tag="selM")
                nc.vector.tensor_tensor(out=selM[:], in0=Mw[:], in1=M[:],
                                        op=mybir.AluOpType.not_equal)
                ids = m_pool.tile([P, C], F32, tag="ids")
                nc.vector.tensor_tensor(out=ids[:], in0=selM[:], in1=ids_iota[:],
                                        op=mybir.AluOpType.mult)
                idsel = s_pool.tile([P, K], F32, tag="idsel")
                for r in range(K // 8):
                    nc.vector.max(idsel[:, r * 8:(r + 1) * 8], ids[:])
                    nc.vector.match_replace(out=ids[:], in_to_replace=idsel[:, r * 8:(r + 1) * 8],
                                            in_values=ids[:], imm_value=0.0)

                # offsets: global chunk index = rowbase - e + b*P*C (exact in f32)
                offs_f = s_pool.tile([P, K], F32, tag="offs_f")
                nc.vector.tensor_scalar(offs_f[:], idsel[:], -1.0, None,
                                        op0=mybir.AluOpType.mult)
                nc.vector.tensor_scalar(offs_f[:], offs_f[:], rowbase[:, :1],
                                        float(b * P * C),
                                        op0=mybir.AluOpType.add, op1=mybir.AluOpType.add)
                offs = s_pool.tile([P, K], I32, tag="offs")
                nc.vector.tensor_copy(offs[:], offs_f[:])

                # gather 32 chunks/row from x. The walrus indirect-DMA lowering
                # supports one dynamic offset per partition and a 2D SBUF side,
                # so issue K gathers of [P, L] each.
                G = g_pool.tile([P, K, L], F32, tag="G")
                for k in range(K):
                    nc.gpsimd.indirect_dma_start(
                        out=G[:, k, :], out_offset=None,
                        in_=x_chunks,
                        in_offset=bass.IndirectOffsetOnAxis(ap=offs[:, k:k + 1], axis=0),
                    )
                if pending_scatter is not None:
                    emit_scatter(pending_scatter)
                    pending_scatter = None
                Gf = G[:].rearrange("p k l -> p (k l)")

                # top-32 elements of G: zap to 0, then masked = Gf - Gw
                # (kept values are > 0 whenever every row has >= 32 positives,
                #  so zapped zeros are never re-extracted)
                Gw = g_pool.tile([P, K * L], F32, tag="Gw")
                gx8 = s_pool.tile([P, 8], F32, tag="gx8")
                gsrc = Gf
                for _ in range(K // 8):
                    nc.vector.max(gx8[:], gsrc)
                    nc.vector.match_replace(out=Gw[:], in_to_replace=gx8[:],
                                            in_values=gsrc, imm_value=0.0)
                    gsrc = Gw[:]
                nc.vector.tensor_tensor(out=Gw[:], in0=Gf, in1=Gw[:],
                                        op=mybir.AluOpType.subtract)

                # scatter masked chunks into pre-zeroed xz, K scatters of [P, L];
                # deferred to after the next block's gathers (see above).
                pending_scatter = (b, offs, Gw[:].rearrange("p (k l) -> p k l", l=L))
            if pending_scatter is not None:
                emit_scatter(pending_scatter)
    nc.finalize()
    return nc


def kernel(x: np.ndarray):
    global LAST_EXEC_TIME_NS, LAST_TRACE_DIR, _CACHED_NC
    x = np.ascontiguousarray(np.asarray(x, dtype=np.float32))
    assert x.shape == (N_ROWS, N_COLS), x.shape
    Rs = N_ROWS // N_CORES

    if _CACHED_NC is None:
        _CACHED_NC = _build(Rs, N_COLS)
    nc = _CACHED_NC

    in_maps = [{"x": x[i * Rs:(i + 1) * Rs]} for i in range(N_CORES)]
    tmpdir = None
    if os.environ.get("BASS_TRACE"):
        import tempfile
        tmpdir = tempfile.mkdtemp(prefix="topk_trace_")
        LAST_TRACE_DIR = tmpdir
    res = run_bass_kernel_spmd(nc, in_maps, core_ids=list(range(N_CORES)),
                               tmpdir=tmpdir)
    LAST_EXEC_TIME_NS = res.exec_time_ns

    y = np.concatenate([np.asarray(res.results[i]["y"]).reshape(Rs, N_COLS)
                        for i in range(N_CORES)], axis=0)
    xz = np.concatenate([np.asarray(res.results[i]["xz"]).reshape(Rs, N_COLS)
                         for i in range(N_CORES)], axis=0)
    return y, xz
